# revision 27
# baseline (speedup 1.0000x reference)
"""Trainium2 Bass kernel for the Clifford-algebra geometric product.

  out[..., j] = sum_{i,k} a[..., i] * cayley[i, j, k] * b[..., k]

Full inputs a, b: (2048, 1024, 8) fp32, cayley: (8, 8, 8) fp32.
Sharding: pure data parallelism over the leading batch axis across 8
NeuronCores.

Fast path ("pauli"): Cl(3,0) is isomorphic to the 2x2 complex matrix
algebra M2(C) (Pauli matrices).  Writing each multivector as
  M = [[ (a0+a3) + i(a12+a123), (a1-a13) + i(a23-a2) ],
       [ (a1+a13) + i(a2+a23),  (a0-a3) + i(a123-a12) ]]
the geometric product is the 2x2 complex matmul C = A*B, and the output
coefficients are (sum/difference pairs of C entries)/2.  This cuts the
elementwise work from 120 ops/position (64 products + 56 tree adds) to
80 ops/position (16 transform + 32 products + 24 combine + 8 out), all
expressible as plain tensor_tensor adds/mults.

All compute runs in fp16 on the DVE at 2 elem/lane/cycle (the 2x_1p
packed mode; fp32 TT runs at 1x).  ScalarE (ACT) does the
interleaved<->plane transposes with the fp32<->fp16 conversion and the
0.5 scale folded in (transposed ACT copies cost the same with or
without scale/convert).  Measured end-to-end fp16 error vs the fp32
reference: ~8e-4 max-rel (gate is 2e-2).

Layout per tile of width w positions/partition:
  ta/tb  [P, w, 8] fp32   interleaved (contiguous DMA)
  tAB    [P, 16, w] fp16  blade planes: [0:8] = a*0.5, [8:16] = b
  tfAB   [P, 16, w] fp16  transformed planes, alpha(r,s,e)=4r+2s+e
                          (A in [0:8]) / beta(s,c,e)=4s+2c+e (B in [8:16])
  tp     [P, 32, w] fp16  products pi(r,s,e1,c,e2)=16r+8s+4e1+2c+e2
  tl     [P, 16, w] fp16  mu(r,s,h,c)=8r+4s+2h+c   (h=0 real, 1 imag)
  tC     [P, 8, w]  fp16  chi(r,e,c)=4r+2e+c
  toP    [P, 8, w]  fp16  output blade planes
  to     [P, w, 8]  fp32  interleaved output (ACT reinterleave)
"""

import sys

if "/opt/trn_rl_repo" not in sys.path:
    sys.path.insert(0, "/opt/trn_rl_repo")

import numpy as np

N_CORES = 8
P = 128  # SBUF partitions
N = 8    # blades

WIDTHS = (64, 192, 320, 384, 384, 384, 256, 64)  # sums to 2048 = npos_local / P

_module_cache = {}


def _canonical_cayley() -> np.ndarray:
    """Cl(3,0) geometric-product table, short-lex blade order (= reference)."""
    import itertools, functools, operator

    metric = [1, 1, 1]
    nv = len(metric)
    n = 2 ** nv
    basis = [1 << k for k in range(nv)]
    combos = itertools.chain.from_iterable(
        itertools.combinations(basis, r) for r in range(nv + 1))
    i2b = [functools.reduce(operator.or_, t, 0) for t in combos]
    b2i = {b: i for i, b in enumerate(i2b)}
    c = np.zeros((n, n, n), dtype=np.float32)
    for i, bi in enumerate(i2b):
        for j, bj in enumerate(i2b):
            a = bi >> 1
            s = 0
            while a:
                s += bin(a & bj).count("1")
                a >>= 1
            sign = -1.0 if (s & 1) else 1.0
            common = bi & bj
            k = 0
            while common:
                if common & 1:
                    sign *= metric[k]
                k += 1
                common >>= 1
            c[i, b2i[bi ^ bj], j] = sign
    return c


# ---------------- pauli fast path ----------------


def build_module_pauli(npos_local: int, widths):
    import concourse.bacc as bacc
    import concourse.mybir as mybir
    import concourse.tile as tile
    from concourse.bass import AP

    assert sum(widths) * P == npos_local
    f16 = mybir.dt.float16
    f32 = mybir.dt.float32
    ADD = mybir.AluOpType.add
    SUB = mybir.AluOpType.subtract
    MUL = mybir.AluOpType.mult

    nc = bacc.Bacc(None, target_bir_lowering=False, debug=False)
    with tile.TileContext(nc) as tc:
        with tc.tile_pool(name="dram", bufs=1, space="DRAM") as dram:
            a = dram.tile((npos_local, N), f32, kind="ExternalInput")
            b = dram.tile((npos_local, N), f32, kind="ExternalInput")
            out = dram.tile((npos_local, N), f32, kind="ExternalOutput")
            av = a[:].rearrange("(p f) n -> p (f n)", p=P)
            bv = b[:].rearrange("(p f) n -> p (f n)", p=P)
            ov = out[:].rearrange("(p f) n -> p (f n)", p=P)

            with (
                tc.tile_pool(name="io", bufs=3) as io_pool,
                tc.tile_pool(name="pl2", bufs=2) as pl2_pool,
                tc.tile_pool(name="pl", bufs=1) as pl_pool,
                tc.tile_pool(name="st", bufs=2) as st_pool,
                tc.tile_pool(name="wrm", bufs=1) as wrm_pool,
            ):
                # prewarm ScalarE's activation table (ACT_TABLE_LOAD ~2.6us)
                # before the first DMA lands so tile-0 deps don't pay for it
                warm = wrm_pool.tile([P, 2], f32, tag="warm")
                nc.vector.memset(warm[:, 0:1], 0.0)
                nc.scalar.copy(out=warm[:, 1:2], in_=warm[:, 0:1])

                def ap(t, pfree, off, dims):
                    base = t[:]
                    return AP(base.tensor, base.offset + off,
                              [[pfree, P]] + dims)

                pending_reint = []  # deferred reint+store closures, 1 tile lag

                pos0 = 0
                for t, w in enumerate(widths):
                    sl = slice(pos0 * N, (pos0 + w) * N)
                    pos0 += w
                    tab = io_pool.tile([P, 2 * w, N], f32, tag="tab")
                    tAB = pl2_pool.tile([P, 8, 2 * w], f16, tag="tAB")
                    tfAB = pl_pool.tile([P, 16, w], f16, tag="tfAB")
                    tl = pl_pool.tile([P, 16, w], f16, tag="tl")
                    tC = pl_pool.tile([P, 8, w], f16, tag="tC")
                    toP = pl2_pool.tile([P, 8, w], f16, tag="toP")

                    nc.sync.dma_start(
                        out=tab[:, 0:w, :].rearrange("p f n -> p (f n)"),
                        in_=av[:, sl])
                    nc.scalar.dma_start(
                        out=tab[:, w:2 * w, :].rearrange("p f n -> p (f n)"),
                        in_=bv[:, sl])

                    # single merged deinterleave + fp16 convert; tile 0 on
                    # DVE (otherwise idle during fill), rest on ScalarE.
                    # blade plane n holds [a_n (w) | b_n (w)]
                    last = t == len(widths) - 1
                    if t == 0:
                        nc.vector.tensor_copy(
                            out=tAB[:], in_=tab[:].transpose([0, 2, 1]))
                    elif last:
                        # pre-scale a by 0.5 here so the reint below is a
                        # plain DVE cast right after OTF (short tail)
                        nc.scalar.mul(
                            out=tAB[:, :, 0:w],
                            in_=tab[:, 0:w, :].transpose([0, 2, 1]), mul=0.5)
                        nc.scalar.copy(
                            out=tAB[:, :, w:2 * w],
                            in_=tab[:, w:2 * w, :].transpose([0, 2, 1]))
                    else:
                        nc.scalar.copy(
                            out=tAB[:], in_=tab[:].transpose([0, 2, 1]))

                    # previous tile's reinterleave goes to ACT *after* this
                    # tile's deint so DVE never waits on ACT at boundaries
                    for fn in pending_reint:
                        fn()
                    pending_reint = []

                    # --- TF: 4 DVE ops [P,2,2,w] over (operand, pair, w) ---
                    # gamma: 0=M00r 1=M00i 2=M01r 3=M01i 4=M10r 5=M10i 6=M11r 7=M11i
                    # in tAB: blade n -> a at n*2w, b at n*2w + w
                    pf = 16 * w
                    for (oo, so, i0, s0, i1, s1, alu) in (
                        (0 * w, 1 * w, 0 * w, 8 * w, 6 * w, 8 * w, ADD),     # g0=m0+m3 g1=m4+m7
                        (4 * w, 1 * w, 2 * w, 2 * w, 10 * w, 2 * w, ADD),    # g4=m1+m5 g5=m2+m6
                        (2 * w, 4 * w, 2 * w, -2 * w, 10 * w, -4 * w, SUB),  # g2=m1-m5 g6=m0-m3
                        (3 * w, 4 * w, 12 * w, 2 * w, 4 * w, 4 * w, SUB),    # g3=m6-m2 g7=m7-m4
                    ):
                        nc.vector.tensor_tensor(
                            out=ap(tfAB, pf, oo,
                                   [[8 * w, 2], [so, 2], [1, w]]),
                            in0=ap(tAB, pf, i0,
                                   [[w, 2], [s0, 2], [1, w]]),
                            in1=ap(tAB, pf, i1,
                                   [[w, 2], [s1, 2], [1, w]]),
                            op=alu)

                    # --- PROD: 4 DVE ops; tp(r,s,e1,c,e2)=16r+8s+4e1+2c+e2 ---
                    tp = pl_pool.tile([P, 32, w], f16, tag="tp")
                    for r in (0, 1):
                        for s in (0, 1):
                            nc.vector.tensor_tensor(
                                out=ap(tp, 32 * w, (16 * r + 8 * s) * w,
                                       [[4 * w, 2], [w, 4], [1, w]]),
                                in0=ap(tfAB, pf, (4 * r + 2 * s) * w,
                                       [[w, 2], [0, 4], [1, w]]),
                                in1=ap(tfAB, pf, (8 + 4 * s) * w,
                                       [[0, 2], [w, 4], [1, w]]),
                                op=MUL)
                    # --- L1: 2 DVE ops [P,4,2,w] over ((r,s), c, w) ---
                    nc.vector.tensor_tensor(  # real: p(..00) - p(..11)
                        out=ap(tl, 16 * w, 0,
                               [[4 * w, 4], [w, 2], [1, w]]),
                        in0=ap(tp, 32 * w, 0,
                               [[8 * w, 4], [2 * w, 2], [1, w]]),
                        in1=ap(tp, 32 * w, 5 * w,
                               [[8 * w, 4], [2 * w, 2], [1, w]]),
                        op=SUB)
                    nc.vector.tensor_tensor(  # imag: p(..01) + p(..10)
                        out=ap(tl, 16 * w, 2 * w,
                               [[4 * w, 4], [w, 2], [1, w]]),
                        in0=ap(tp, 32 * w, 1 * w,
                               [[8 * w, 4], [2 * w, 2], [1, w]]),
                        in1=ap(tp, 32 * w, 4 * w,
                               [[8 * w, 4], [2 * w, 2], [1, w]]),
                        op=ADD)

                    # --- L2: 1 DVE op [P,2,4,w] over (r, (e,c), w) ---
                    nc.vector.tensor_tensor(
                        out=ap(tC, 8 * w, 0, [[4 * w, 2], [w, 4], [1, w]]),
                        in0=ap(tl, 16 * w, 0, [[8 * w, 2], [w, 4], [1, w]]),
                        in1=ap(tl, 16 * w, 4 * w,
                               [[8 * w, 2], [w, 4], [1, w]]),
                        op=ADD)

                    # --- OTF: 3 DVE ops -> blade planes ---
                    # o0=C0+C5 o1=C1+C4 o6=C6+C3 o7=C7+C2
                    nc.vector.tensor_tensor(
                        out=ap(toP, 8 * w, 0, [[6 * w, 2], [w, 2], [1, w]]),
                        in0=ap(tC, 8 * w, 0, [[6 * w, 2], [w, 2], [1, w]]),
                        in1=ap(tC, 8 * w, 5 * w,
                               [[-2 * w, 2], [-w, 2], [1, w]]),
                        op=ADD)
                    # o2=C6-C3 o3=C0-C5
                    nc.vector.tensor_tensor(
                        out=ap(toP, 8 * w, 2 * w, [[w, 2], [1, w]]),
                        in0=ap(tC, 8 * w, 6 * w, [[-6 * w, 2], [1, w]]),
                        in1=ap(tC, 8 * w, 3 * w, [[2 * w, 2], [1, w]]),
                        op=SUB)
                    # o4=C2-C7 o5=C4-C1
                    nc.vector.tensor_tensor(
                        out=ap(toP, 8 * w, 4 * w, [[w, 2], [1, w]]),
                        in0=ap(tC, 8 * w, 2 * w, [[2 * w, 2], [1, w]]),
                        in1=ap(tC, 8 * w, 7 * w, [[-6 * w, 2], [1, w]]),
                        op=SUB)

                    # reinterleave halves + fp32 convert + 0.5 scale on
                    # ScalarE; deferred one tile so ACT runs deint first
                    def make_reint(toP=toP, sl=sl, w=w, last=last):
                        def emit():
                            h = w // 2
                            for (c0, cn) in ((0, h), (h, w - h)):
                                toH = st_pool.tile([P, cn, N], f32,
                                                   tag=f"to{c0 != 0:d}")
                                src = toP[:, :, c0:c0 + cn].transpose(
                                    [0, 2, 1])
                                if last:  # input already scaled: plain cast
                                    nc.vector.tensor_copy(out=toH[:], in_=src)
                                else:
                                    nc.scalar.mul(out=toH[:], in_=src,
                                                  mul=0.5)
                                nc.sync.dma_start(
                                    out=ov[:, sl][:, c0 * N:(c0 + cn) * N],
                                    in_=toH[:].rearrange("p f n -> p (f n)"))
                        return emit

                    if t == len(widths) - 1:
                        make_reint()()
                    else:
                        pending_reint.append(make_reint())
                for fn in pending_reint:
                    fn()
    nc.compile()
    return nc, a.name, b.name, out.name


# ---------------- generic fallback (any cayley) ----------------


def _terms_by_j(cayley: np.ndarray):
    terms = [[] for _ in range(N)]
    for i in range(N):
        for j in range(N):
            for k in range(N):
                v = float(cayley[i, j, k])
                if v != 0.0:
                    terms[j].append((i, k, v))
    return terms


def _build_module(npos_local: int, terms):
    import concourse.bacc as bacc
    import concourse.mybir as mybir
    import concourse.tile as tile

    W = 256
    assert npos_local % (P * W) == 0
    T = npos_local // (P * W)
    fast = all(len(t) == 8 for t in terms)

    nc = bacc.Bacc(None, target_bir_lowering=False, debug=False)
    with tile.TileContext(nc) as tc:
        with tc.tile_pool(name="dram", bufs=1, space="DRAM") as dram:
            a = dram.tile((npos_local, N), mybir.dt.float32, kind="ExternalInput")
            b = dram.tile((npos_local, N), mybir.dt.float32, kind="ExternalInput")
            out = dram.tile((npos_local, N), mybir.dt.float32, kind="ExternalOutput")
            av = a[:].rearrange("(p f) n -> p (f n)", p=P)
            bv = b[:].rearrange("(p f) n -> p (f n)", p=P)
            ov = out[:].rearrange("(p f) n -> p (f n)", p=P)
            with (
                tc.tile_pool(name="io", bufs=4) as io_pool,
                tc.tile_pool(name="prod", bufs=1) as prod_pool,
            ):
                for t in range(T):
                    sl = slice(t * W * N, (t + 1) * W * N)
                    ta = io_pool.tile([P, W, N], mybir.dt.float32, tag="ta")
                    tb = io_pool.tile([P, W, N], mybir.dt.float32, tag="tb")
                    to = io_pool.tile([P, W, N], mybir.dt.float32, tag="to")
                    nc.sync.dma_start(
                        out=ta[:].rearrange("p f n -> p (f n)"), in_=av[:, sl]
                    )
                    nc.sync.dma_start(
                        out=tb[:].rearrange("p f n -> p (f n)"), in_=bv[:, sl]
                    )
                    if fast:
                        p0 = prod_pool.tile([P, 64, W], mybir.dt.float32, tag="p0")
                        p1 = prod_pool.tile([P, 32, W], mybir.dt.float32, tag="p1")
                        p2 = prod_pool.tile([P, 16, W], mybir.dt.float32, tag="p2")
                        for j in range(N):
                            for l, (i, k, v) in enumerate(terms[j]):
                                nc.vector.scalar_tensor_tensor(
                                    out=p0[:, j * 8 + l, :],
                                    in0=ta[:, :, i],
                                    scalar=v,
                                    in1=tb[:, :, k],
                                    op0=mybir.AluOpType.mult,
                                    op1=mybir.AluOpType.mult,
                                )
                        nc.vector.tensor_tensor(
                            out=p1[:], in0=p0[:, 0::2, :], in1=p0[:, 1::2, :],
                            op=mybir.AluOpType.add,
                        )
                        nc.vector.tensor_tensor(
                            out=p2[:], in0=p1[:, 0::2, :], in1=p1[:, 1::2, :],
                            op=mybir.AluOpType.add,
                        )
                        nc.vector.tensor_tensor(
                            out=to[:].transpose([0, 2, 1]),
                            in0=p2[:, 0::2, :], in1=p2[:, 1::2, :],
                            op=mybir.AluOpType.add,
                        )
                    else:
                        pa = prod_pool.tile([P, W], mybir.dt.float32, tag="pa")
                        acc = prod_pool.tile([P, W], mybir.dt.float32, tag="acc")
                        for j in range(N):
                            if not terms[j]:
                                nc.vector.memset(to[:, :, j], 0.0)
                                continue
                            i, k, v = terms[j][0]
                            nc.vector.scalar_tensor_tensor(
                                out=acc[:], in0=ta[:, :, i], scalar=v,
                                in1=tb[:, :, k],
                                op0=mybir.AluOpType.mult, op1=mybir.AluOpType.mult,
                            )
                            for (i, k, v) in terms[j][1:]:
                                nc.vector.scalar_tensor_tensor(
                                    out=pa[:], in0=ta[:, :, i], scalar=v,
                                    in1=tb[:, :, k],
                                    op0=mybir.AluOpType.mult, op1=mybir.AluOpType.mult,
                                )
                                nc.vector.tensor_tensor(
                                    out=acc[:], in0=acc[:], in1=pa[:],
                                    op=mybir.AluOpType.add,
                                )
                            nc.vector.tensor_copy(out=to[:, :, j], in_=acc[:])
                    nc.sync.dma_start(
                        out=ov[:, sl], in_=to[:].rearrange("p f n -> p (f n)")
                    )
    nc.compile()
    return nc, a.name, b.name, out.name


def _get_module(npos_local: int, cayley: np.ndarray):
    key = (npos_local, cayley.tobytes())
    if key not in _module_cache:
        if (npos_local % P == 0 and sum(WIDTHS) * P == npos_local
                and np.array_equal(cayley, _canonical_cayley())):
            _module_cache[key] = build_module_pauli(npos_local, WIDTHS)
        else:
            _module_cache[key] = _build_module(npos_local, _terms_by_j(cayley))
    return _module_cache[key]


def _run(inputs: dict, trace: bool = False, tmpdir=None):
    a = np.asarray(inputs["a"], dtype=np.float32)
    b = np.asarray(inputs["b"], dtype=np.float32)
    cayley = np.asarray(inputs["cayley"], dtype=np.float32)
    B, S, NN = a.shape
    assert NN == N and b.shape == a.shape and cayley.shape == (N, N, N)
    assert B % N_CORES == 0
    nb = B // N_CORES
    npos_local = nb * S

    nc, a_name, b_name, out_name = _get_module(npos_local, cayley)

    a_sh = a.reshape(N_CORES, npos_local, N)
    b_sh = b.reshape(N_CORES, npos_local, N)
    in_maps = [
        {a_name: np.ascontiguousarray(a_sh[c]), b_name: np.ascontiguousarray(b_sh[c])}
        for c in range(N_CORES)
    ]

    from concourse import bass_utils

    kwargs = {}
    if trace:
        _install_ntff_shim()
        bass_utils.upload_artifacts = lambda d: f"local:{d}"
        kwargs = {"trace": True, "tmpdir": tmpdir}
    res = bass_utils.run_bass_kernel_spmd(
        nc, in_maps, core_ids=list(range(N_CORES)), **kwargs
    )
    out = np.concatenate(
        [res.results[c][out_name].reshape(1, nb, S, N) for c in range(N_CORES)], axis=0
    ).reshape(B, S, N)
    return out, res


def kernel(**inputs) -> np.ndarray:
    out, _ = _run(inputs, trace=False)
    return out


def kernel_traced(**inputs):
    """Run with NTFF profiling; returns (out, exec_time_ns, trace_path)."""
    import tempfile

    out, res = _run(inputs, trace=True, tmpdir=tempfile.mkdtemp(prefix="gp_trace_"))
    trace_path = res.instructions_and_trace[1] if res.instructions_and_trace else None
    return out, res.exec_time_ns, trace_path


def _install_ntff_shim():
    """Provide antenv.axon_hooks with an NTFF profile hook if missing."""
    try:
        from antenv.axon_hooks import get_axon_ntff_profile_hook  # noqa: F401

        return
    except ImportError:
        pass
    import types, ctypes, contextlib

    holder = {"hook": None}
    mod = types.ModuleType("antenv.axon_hooks")
    mod.set_axon_ntff_profile_hook = lambda h: holder.__setitem__("hook", h)
    mod.get_axon_ntff_profile_hook = lambda: holder["hook"]
    sys.modules["antenv.axon_hooks"] = mod

    so_path = "/opt/axon/libaxon_pjrt.so"
    try:
        lib = ctypes.CDLL(so_path)
        if not hasattr(lib, "axon_start_nrt_profile"):
            return
    except OSError:
        return
    lib.axon_start_nrt_profile.argtypes = [
        ctypes.POINTER(ctypes.c_int64),
        ctypes.c_size_t,
    ]
    lib.axon_start_nrt_profile.restype = ctypes.c_int64
    lib.axon_stop_nrt_profile.argtypes = [ctypes.c_char_p]
    lib.axon_stop_nrt_profile.restype = ctypes.c_int64

    @contextlib.contextmanager
    def _hook(output_dir, device_ids):
        import jax

        jax.devices()
        if device_ids:
            ids = (ctypes.c_int64 * len(device_ids))(*device_ids)
            rc = lib.axon_start_nrt_profile(ids, len(device_ids))
        else:
            rc = lib.axon_start_nrt_profile(None, 0)
        if rc != 0:
            raise RuntimeError(f"axon_start_nrt_profile rc={rc}")
        try:
            yield
        finally:
            n = lib.axon_stop_nrt_profile(str(output_dir).encode())
            print(f"profile: {n} file(s) written to {output_dir}", file=sys.stderr)

    mod.set_axon_ntff_profile_hook(_hook)


# revision 28
# speedup vs baseline: 1.2346x; 1.2346x over previous
"""Trainium2 Bass kernel for the Clifford-algebra geometric product.

  out[..., j] = sum_{i,k} a[..., i] * cayley[i, j, k] * b[..., k]

Full inputs a, b: (2048, 1024, 8) fp32, cayley: (8, 8, 8) fp32.
Sharding: pure data parallelism over the leading batch axis across 8
NeuronCores.

Fast path ("pauli"): Cl(3,0) is isomorphic to the 2x2 complex matrix
algebra M2(C) (Pauli matrices).  Writing each multivector as
  M = [[ (a0+a3) + i(a12+a123), (a1-a13) + i(a23-a2) ],
       [ (a1+a13) + i(a2+a23),  (a0-a3) + i(a123-a12) ]]
the geometric product is the 2x2 complex matmul C = A*B, and the output
coefficients are (sum/difference pairs of C entries)/2.  This cuts the
elementwise work from 120 ops/position (64 products + 56 tree adds) to
80 ops/position (16 transform + 32 products + 24 combine + 8 out), all
expressible as plain tensor_tensor adds/mults.

All compute runs in fp16 on the DVE at 2 elem/lane/cycle (the 2x_1p
packed mode; fp32 TT runs at 1x).  ScalarE (ACT) does the
interleaved<->plane transposes with the fp32<->fp16 conversion and the
0.5 scale folded in (transposed ACT copies cost the same with or
without scale/convert).  Measured end-to-end fp16 error vs the fp32
reference: ~8e-4 max-rel (gate is 2e-2).

Layout per tile of width w positions/partition:
  ta/tb  [P, w, 8] fp32   interleaved (contiguous DMA)
  tAB    [P, 16, w] fp16  blade planes: [0:8] = a*0.5, [8:16] = b
  tfAB   [P, 16, w] fp16  transformed planes, alpha(r,s,e)=4r+2s+e
                          (A in [0:8]) / beta(s,c,e)=4s+2c+e (B in [8:16])
  tp     [P, 32, w] fp16  products pi(r,s,e1,c,e2)=16r+8s+4e1+2c+e2
  tl     [P, 16, w] fp16  mu(r,s,h,c)=8r+4s+2h+c   (h=0 real, 1 imag)
  tC     [P, 8, w]  fp16  chi(r,e,c)=4r+2e+c
  toP    [P, 8, w]  fp16  output blade planes
  to     [P, w, 8]  fp32  interleaved output (ACT reinterleave)
"""

import sys

if "/opt/trn_rl_repo" not in sys.path:
    sys.path.insert(0, "/opt/trn_rl_repo")

import numpy as np

N_CORES = 8
P = 128  # SBUF partitions
N = 8    # blades

WIDTHS = (64, 192, 320, 384, 384, 384, 256, 64)  # sums to 2048 = npos_local / P

_module_cache = {}


def _canonical_cayley() -> np.ndarray:
    """Cl(3,0) geometric-product table, short-lex blade order (= reference)."""
    import itertools, functools, operator

    metric = [1, 1, 1]
    nv = len(metric)
    n = 2 ** nv
    basis = [1 << k for k in range(nv)]
    combos = itertools.chain.from_iterable(
        itertools.combinations(basis, r) for r in range(nv + 1))
    i2b = [functools.reduce(operator.or_, t, 0) for t in combos]
    b2i = {b: i for i, b in enumerate(i2b)}
    c = np.zeros((n, n, n), dtype=np.float32)
    for i, bi in enumerate(i2b):
        for j, bj in enumerate(i2b):
            a = bi >> 1
            s = 0
            while a:
                s += bin(a & bj).count("1")
                a >>= 1
            sign = -1.0 if (s & 1) else 1.0
            common = bi & bj
            k = 0
            while common:
                if common & 1:
                    sign *= metric[k]
                k += 1
                common >>= 1
            c[i, b2i[bi ^ bj], j] = sign
    return c


# ---------------- pauli fast path ----------------


def build_module_pauli(npos_local: int, widths):
    import concourse.bacc as bacc
    import concourse.mybir as mybir
    import concourse.tile as tile
    from concourse.bass import AP

    assert sum(widths) * P == npos_local
    f16 = mybir.dt.float16
    f32 = mybir.dt.float32
    ADD = mybir.AluOpType.add
    SUB = mybir.AluOpType.subtract
    MUL = mybir.AluOpType.mult

    nc = bacc.Bacc(None, target_bir_lowering=False, debug=False)
    with tile.TileContext(nc) as tc:
        with tc.tile_pool(name="dram", bufs=1, space="DRAM") as dram:
            a = dram.tile((npos_local, N), f32, kind="ExternalInput")
            b = dram.tile((npos_local, N), f32, kind="ExternalInput")
            out = dram.tile((npos_local, N), f32, kind="ExternalOutput")
            av = a[:].rearrange("(p f) n -> p (f n)", p=P)
            bv = b[:].rearrange("(p f) n -> p (f n)", p=P)
            ov = out[:].rearrange("(p f) n -> p (f n)", p=P)

            with (
                tc.tile_pool(name="io", bufs=3) as io_pool,
                tc.tile_pool(name="pl2", bufs=2) as pl2_pool,
                tc.tile_pool(name="pl", bufs=1) as pl_pool,
                tc.tile_pool(name="st", bufs=2) as st_pool,
                tc.tile_pool(name="wrm", bufs=1) as wrm_pool,
            ):
                # prewarm ScalarE's activation table (ACT_TABLE_LOAD ~2.6us)
                # before the first DMA lands so tile-0 deps don't pay for it
                warm = wrm_pool.tile([P, 2], f32, tag="warm")
                nc.vector.memset(warm[:, 0:1], 0.0)
                nc.scalar.copy(out=warm[:, 1:2], in_=warm[:, 0:1])

                def ap(t, pfree, off, dims):
                    base = t[:]
                    return AP(base.tensor, base.offset + off,
                              [[pfree, P]] + dims)

                pending_reint = []  # deferred reint+store closures, 1 tile lag

                pos0 = 0
                for t, w in enumerate(widths):
                    sl = slice(pos0 * N, (pos0 + w) * N)
                    pos0 += w
                    tab = io_pool.tile([P, 2 * w, N], f32, tag="tab")
                    tAB = pl2_pool.tile([P, 8, 2 * w], f16, tag="tAB")
                    tfAB = pl_pool.tile([P, 16, w], f16, tag="tfAB")
                    tl = pl_pool.tile([P, 16, w], f16, tag="tl")
                    tC = pl_pool.tile([P, 8, w], f16, tag="tC")
                    toP = pl2_pool.tile([P, 8, w], f16, tag="toP")

                    nc.sync.dma_start(
                        out=tab[:, 0:w, :].rearrange("p f n -> p (f n)"),
                        in_=av[:, sl])
                    nc.scalar.dma_start(
                        out=tab[:, w:2 * w, :].rearrange("p f n -> p (f n)"),
                        in_=bv[:, sl])

                    # single merged deinterleave + fp16 convert; tile 0 on
                    # DVE (otherwise idle during fill), rest on ScalarE.
                    # blade plane n holds [a_n (w) | b_n (w)]
                    last = t == len(widths) - 1
                    if t == 0:
                        # unscaled DVE deint during the fill; t0's reint
                        # carries the 0.5 on ACT instead
                        nc.vector.tensor_copy(
                            out=tAB[:], in_=tab[:].transpose([0, 2, 1]))
                    else:
                        # pre-scale a by 0.5 at the deint so every reint is
                        # a pure cast runnable on either engine
                        nc.scalar.mul(
                            out=tAB[:, :, 0:w],
                            in_=tab[:, 0:w, :].transpose([0, 2, 1]), mul=0.5)
                        nc.scalar.copy(
                            out=tAB[:, :, w:2 * w],
                            in_=tab[:, w:2 * w, :].transpose([0, 2, 1]))

                    # previous tile's reinterleave goes to ACT *after* this
                    # tile's deint so DVE never waits on ACT at boundaries
                    for fn in pending_reint:
                        fn()
                    pending_reint = []

                    # --- TF: 4 DVE ops [P,2,2,w] over (operand, pair, w) ---
                    # gamma: 0=M00r 1=M00i 2=M01r 3=M01i 4=M10r 5=M10i 6=M11r 7=M11i
                    # in tAB: blade n -> a at n*2w, b at n*2w + w
                    pf = 16 * w
                    for (oo, so, i0, s0, i1, s1, alu) in (
                        (0 * w, 1 * w, 0 * w, 8 * w, 6 * w, 8 * w, ADD),     # g0=m0+m3 g1=m4+m7
                        (4 * w, 1 * w, 2 * w, 2 * w, 10 * w, 2 * w, ADD),    # g4=m1+m5 g5=m2+m6
                        (2 * w, 4 * w, 2 * w, -2 * w, 10 * w, -4 * w, SUB),  # g2=m1-m5 g6=m0-m3
                        (3 * w, 4 * w, 12 * w, 2 * w, 4 * w, 4 * w, SUB),    # g3=m6-m2 g7=m7-m4
                    ):
                        nc.vector.tensor_tensor(
                            out=ap(tfAB, pf, oo,
                                   [[8 * w, 2], [so, 2], [1, w]]),
                            in0=ap(tAB, pf, i0,
                                   [[w, 2], [s0, 2], [1, w]]),
                            in1=ap(tAB, pf, i1,
                                   [[w, 2], [s1, 2], [1, w]]),
                            op=alu)

                    # --- PROD: 4 DVE ops; tp(r,s,e1,c,e2)=16r+8s+4e1+2c+e2 ---
                    tp = pl_pool.tile([P, 32, w], f16, tag="tp")
                    for r in (0, 1):
                        for s in (0, 1):
                            nc.vector.tensor_tensor(
                                out=ap(tp, 32 * w, (16 * r + 8 * s) * w,
                                       [[4 * w, 2], [w, 4], [1, w]]),
                                in0=ap(tfAB, pf, (4 * r + 2 * s) * w,
                                       [[w, 2], [0, 4], [1, w]]),
                                in1=ap(tfAB, pf, (8 + 4 * s) * w,
                                       [[0, 2], [w, 4], [1, w]]),
                                op=MUL)
                    # --- L1: 2 DVE ops [P,4,2,w] over ((r,s), c, w) ---
                    nc.vector.tensor_tensor(  # real: p(..00) - p(..11)
                        out=ap(tl, 16 * w, 0,
                               [[4 * w, 4], [w, 2], [1, w]]),
                        in0=ap(tp, 32 * w, 0,
                               [[8 * w, 4], [2 * w, 2], [1, w]]),
                        in1=ap(tp, 32 * w, 5 * w,
                               [[8 * w, 4], [2 * w, 2], [1, w]]),
                        op=SUB)
                    nc.vector.tensor_tensor(  # imag: p(..01) + p(..10)
                        out=ap(tl, 16 * w, 2 * w,
                               [[4 * w, 4], [w, 2], [1, w]]),
                        in0=ap(tp, 32 * w, 1 * w,
                               [[8 * w, 4], [2 * w, 2], [1, w]]),
                        in1=ap(tp, 32 * w, 4 * w,
                               [[8 * w, 4], [2 * w, 2], [1, w]]),
                        op=ADD)

                    # --- L2: 1 DVE op [P,2,4,w] over (r, (e,c), w) ---
                    nc.vector.tensor_tensor(
                        out=ap(tC, 8 * w, 0, [[4 * w, 2], [w, 4], [1, w]]),
                        in0=ap(tl, 16 * w, 0, [[8 * w, 2], [w, 4], [1, w]]),
                        in1=ap(tl, 16 * w, 4 * w,
                               [[8 * w, 2], [w, 4], [1, w]]),
                        op=ADD)

                    # --- OTF: 3 DVE ops -> blade planes ---
                    # o0=C0+C5 o1=C1+C4 o6=C6+C3 o7=C7+C2
                    nc.vector.tensor_tensor(
                        out=ap(toP, 8 * w, 0, [[6 * w, 2], [w, 2], [1, w]]),
                        in0=ap(tC, 8 * w, 0, [[6 * w, 2], [w, 2], [1, w]]),
                        in1=ap(tC, 8 * w, 5 * w,
                               [[-2 * w, 2], [-w, 2], [1, w]]),
                        op=ADD)
                    # o2=C6-C3 o3=C0-C5
                    nc.vector.tensor_tensor(
                        out=ap(toP, 8 * w, 2 * w, [[w, 2], [1, w]]),
                        in0=ap(tC, 8 * w, 6 * w, [[-6 * w, 2], [1, w]]),
                        in1=ap(tC, 8 * w, 3 * w, [[2 * w, 2], [1, w]]),
                        op=SUB)
                    # o4=C2-C7 o5=C4-C1
                    nc.vector.tensor_tensor(
                        out=ap(toP, 8 * w, 4 * w, [[w, 2], [1, w]]),
                        in0=ap(tC, 8 * w, 2 * w, [[2 * w, 2], [1, w]]),
                        in1=ap(tC, 8 * w, 7 * w, [[-6 * w, 2], [1, w]]),
                        op=SUB)

                    # reinterleave halves + fp32 convert + 0.5 scale on
                    # ScalarE; deferred one tile so ACT runs deint first
                    # reints: t0 = scaled on ACT (its deint was unscaled);
                    # t1 and the two tail tiles = pure DVE casts in windows
                    # where DVE would idle; the rest = pure casts on ACT
                    on_dve = t in (1, len(widths) - 2, len(widths) - 1)

                    def make_reint(toP=toP, sl=sl, w=w, t=t, on_dve=on_dve):
                        def emit():
                            h = w // 2
                            for (c0, cn) in ((0, h), (h, w - h)):
                                toH = st_pool.tile([P, cn, N], f32,
                                                   tag=f"to{c0 != 0:d}")
                                src = toP[:, :, c0:c0 + cn].transpose(
                                    [0, 2, 1])
                                if t == 0:
                                    nc.scalar.mul(out=toH[:], in_=src,
                                                  mul=0.5)
                                elif on_dve:
                                    nc.vector.tensor_copy(out=toH[:], in_=src)
                                else:
                                    nc.scalar.copy(out=toH[:], in_=src)
                                nc.sync.dma_start(
                                    out=ov[:, sl][:, c0 * N:(c0 + cn) * N],
                                    in_=toH[:].rearrange("p f n -> p (f n)"))
                        return emit

                    if t == len(widths) - 1:
                        make_reint()()
                    else:
                        pending_reint.append(make_reint())
                for fn in pending_reint:
                    fn()
    nc.compile()
    return nc, a.name, b.name, out.name


# ---------------- generic fallback (any cayley) ----------------


def _terms_by_j(cayley: np.ndarray):
    terms = [[] for _ in range(N)]
    for i in range(N):
        for j in range(N):
            for k in range(N):
                v = float(cayley[i, j, k])
                if v != 0.0:
                    terms[j].append((i, k, v))
    return terms


def _build_module(npos_local: int, terms):
    import concourse.bacc as bacc
    import concourse.mybir as mybir
    import concourse.tile as tile

    W = 256
    assert npos_local % (P * W) == 0
    T = npos_local // (P * W)
    fast = all(len(t) == 8 for t in terms)

    nc = bacc.Bacc(None, target_bir_lowering=False, debug=False)
    with tile.TileContext(nc) as tc:
        with tc.tile_pool(name="dram", bufs=1, space="DRAM") as dram:
            a = dram.tile((npos_local, N), mybir.dt.float32, kind="ExternalInput")
            b = dram.tile((npos_local, N), mybir.dt.float32, kind="ExternalInput")
            out = dram.tile((npos_local, N), mybir.dt.float32, kind="ExternalOutput")
            av = a[:].rearrange("(p f) n -> p (f n)", p=P)
            bv = b[:].rearrange("(p f) n -> p (f n)", p=P)
            ov = out[:].rearrange("(p f) n -> p (f n)", p=P)
            with (
                tc.tile_pool(name="io", bufs=4) as io_pool,
                tc.tile_pool(name="prod", bufs=1) as prod_pool,
            ):
                for t in range(T):
                    sl = slice(t * W * N, (t + 1) * W * N)
                    ta = io_pool.tile([P, W, N], mybir.dt.float32, tag="ta")
                    tb = io_pool.tile([P, W, N], mybir.dt.float32, tag="tb")
                    to = io_pool.tile([P, W, N], mybir.dt.float32, tag="to")
                    nc.sync.dma_start(
                        out=ta[:].rearrange("p f n -> p (f n)"), in_=av[:, sl]
                    )
                    nc.sync.dma_start(
                        out=tb[:].rearrange("p f n -> p (f n)"), in_=bv[:, sl]
                    )
                    if fast:
                        p0 = prod_pool.tile([P, 64, W], mybir.dt.float32, tag="p0")
                        p1 = prod_pool.tile([P, 32, W], mybir.dt.float32, tag="p1")
                        p2 = prod_pool.tile([P, 16, W], mybir.dt.float32, tag="p2")
                        for j in range(N):
                            for l, (i, k, v) in enumerate(terms[j]):
                                nc.vector.scalar_tensor_tensor(
                                    out=p0[:, j * 8 + l, :],
                                    in0=ta[:, :, i],
                                    scalar=v,
                                    in1=tb[:, :, k],
                                    op0=mybir.AluOpType.mult,
                                    op1=mybir.AluOpType.mult,
                                )
                        nc.vector.tensor_tensor(
                            out=p1[:], in0=p0[:, 0::2, :], in1=p0[:, 1::2, :],
                            op=mybir.AluOpType.add,
                        )
                        nc.vector.tensor_tensor(
                            out=p2[:], in0=p1[:, 0::2, :], in1=p1[:, 1::2, :],
                            op=mybir.AluOpType.add,
                        )
                        nc.vector.tensor_tensor(
                            out=to[:].transpose([0, 2, 1]),
                            in0=p2[:, 0::2, :], in1=p2[:, 1::2, :],
                            op=mybir.AluOpType.add,
                        )
                    else:
                        pa = prod_pool.tile([P, W], mybir.dt.float32, tag="pa")
                        acc = prod_pool.tile([P, W], mybir.dt.float32, tag="acc")
                        for j in range(N):
                            if not terms[j]:
                                nc.vector.memset(to[:, :, j], 0.0)
                                continue
                            i, k, v = terms[j][0]
                            nc.vector.scalar_tensor_tensor(
                                out=acc[:], in0=ta[:, :, i], scalar=v,
                                in1=tb[:, :, k],
                                op0=mybir.AluOpType.mult, op1=mybir.AluOpType.mult,
                            )
                            for (i, k, v) in terms[j][1:]:
                                nc.vector.scalar_tensor_tensor(
                                    out=pa[:], in0=ta[:, :, i], scalar=v,
                                    in1=tb[:, :, k],
                                    op0=mybir.AluOpType.mult, op1=mybir.AluOpType.mult,
                                )
                                nc.vector.tensor_tensor(
                                    out=acc[:], in0=acc[:], in1=pa[:],
                                    op=mybir.AluOpType.add,
                                )
                            nc.vector.tensor_copy(out=to[:, :, j], in_=acc[:])
                    nc.sync.dma_start(
                        out=ov[:, sl], in_=to[:].rearrange("p f n -> p (f n)")
                    )
    nc.compile()
    return nc, a.name, b.name, out.name


def _get_module(npos_local: int, cayley: np.ndarray):
    key = (npos_local, cayley.tobytes())
    if key not in _module_cache:
        if (npos_local % P == 0 and sum(WIDTHS) * P == npos_local
                and np.array_equal(cayley, _canonical_cayley())):
            _module_cache[key] = build_module_pauli(npos_local, WIDTHS)
        else:
            _module_cache[key] = _build_module(npos_local, _terms_by_j(cayley))
    return _module_cache[key]


def _run(inputs: dict, trace: bool = False, tmpdir=None):
    a = np.asarray(inputs["a"], dtype=np.float32)
    b = np.asarray(inputs["b"], dtype=np.float32)
    cayley = np.asarray(inputs["cayley"], dtype=np.float32)
    B, S, NN = a.shape
    assert NN == N and b.shape == a.shape and cayley.shape == (N, N, N)
    assert B % N_CORES == 0
    nb = B // N_CORES
    npos_local = nb * S

    nc, a_name, b_name, out_name = _get_module(npos_local, cayley)

    a_sh = a.reshape(N_CORES, npos_local, N)
    b_sh = b.reshape(N_CORES, npos_local, N)
    in_maps = [
        {a_name: np.ascontiguousarray(a_sh[c]), b_name: np.ascontiguousarray(b_sh[c])}
        for c in range(N_CORES)
    ]

    from concourse import bass_utils

    kwargs = {}
    if trace:
        _install_ntff_shim()
        bass_utils.upload_artifacts = lambda d: f"local:{d}"
        kwargs = {"trace": True, "tmpdir": tmpdir}
    res = bass_utils.run_bass_kernel_spmd(
        nc, in_maps, core_ids=list(range(N_CORES)), **kwargs
    )
    out = np.concatenate(
        [res.results[c][out_name].reshape(1, nb, S, N) for c in range(N_CORES)], axis=0
    ).reshape(B, S, N)
    return out, res


def kernel(**inputs) -> np.ndarray:
    out, _ = _run(inputs, trace=False)
    return out


def kernel_traced(**inputs):
    """Run with NTFF profiling; returns (out, exec_time_ns, trace_path)."""
    import tempfile

    out, res = _run(inputs, trace=True, tmpdir=tempfile.mkdtemp(prefix="gp_trace_"))
    trace_path = res.instructions_and_trace[1] if res.instructions_and_trace else None
    return out, res.exec_time_ns, trace_path


def _install_ntff_shim():
    """Provide antenv.axon_hooks with an NTFF profile hook if missing."""
    try:
        from antenv.axon_hooks import get_axon_ntff_profile_hook  # noqa: F401

        return
    except ImportError:
        pass
    import types, ctypes, contextlib

    holder = {"hook": None}
    mod = types.ModuleType("antenv.axon_hooks")
    mod.set_axon_ntff_profile_hook = lambda h: holder.__setitem__("hook", h)
    mod.get_axon_ntff_profile_hook = lambda: holder["hook"]
    sys.modules["antenv.axon_hooks"] = mod

    so_path = "/opt/axon/libaxon_pjrt.so"
    try:
        lib = ctypes.CDLL(so_path)
        if not hasattr(lib, "axon_start_nrt_profile"):
            return
    except OSError:
        return
    lib.axon_start_nrt_profile.argtypes = [
        ctypes.POINTER(ctypes.c_int64),
        ctypes.c_size_t,
    ]
    lib.axon_start_nrt_profile.restype = ctypes.c_int64
    lib.axon_stop_nrt_profile.argtypes = [ctypes.c_char_p]
    lib.axon_stop_nrt_profile.restype = ctypes.c_int64

    @contextlib.contextmanager
    def _hook(output_dir, device_ids):
        import jax

        jax.devices()
        if device_ids:
            ids = (ctypes.c_int64 * len(device_ids))(*device_ids)
            rc = lib.axon_start_nrt_profile(ids, len(device_ids))
        else:
            rc = lib.axon_start_nrt_profile(None, 0)
        if rc != 0:
            raise RuntimeError(f"axon_start_nrt_profile rc={rc}")
        try:
            yield
        finally:
            n = lib.axon_stop_nrt_profile(str(output_dir).encode())
            print(f"profile: {n} file(s) written to {output_dir}", file=sys.stderr)

    mod.set_axon_ntff_profile_hook(_hook)


# revision 29
# speedup vs baseline: 1.2406x; 1.0049x over previous
"""Trainium2 Bass kernel for the Clifford-algebra geometric product.

  out[..., j] = sum_{i,k} a[..., i] * cayley[i, j, k] * b[..., k]

Full inputs a, b: (2048, 1024, 8) fp32, cayley: (8, 8, 8) fp32.
Sharding: pure data parallelism over the leading batch axis across 8
NeuronCores.

Fast path ("pauli"): Cl(3,0) is isomorphic to the 2x2 complex matrix
algebra M2(C) (Pauli matrices).  Writing each multivector as
  M = [[ (a0+a3) + i(a12+a123), (a1-a13) + i(a23-a2) ],
       [ (a1+a13) + i(a2+a23),  (a0-a3) + i(a123-a12) ]]
the geometric product is the 2x2 complex matmul C = A*B, and the output
coefficients are (sum/difference pairs of C entries)/2.  This cuts the
elementwise work from 120 ops/position (64 products + 56 tree adds) to
80 ops/position (16 transform + 32 products + 24 combine + 8 out), all
expressible as plain tensor_tensor adds/mults.

All compute runs in fp16 on the DVE at 2 elem/lane/cycle (the 2x_1p
packed mode; fp32 TT runs at 1x).  ScalarE (ACT) does the
interleaved<->plane transposes with the fp32<->fp16 conversion and the
0.5 scale folded in (transposed ACT copies cost the same with or
without scale/convert).  Measured end-to-end fp16 error vs the fp32
reference: ~8e-4 max-rel (gate is 2e-2).

Layout per tile of width w positions/partition:
  ta/tb  [P, w, 8] fp32   interleaved (contiguous DMA)
  tAB    [P, 16, w] fp16  blade planes: [0:8] = a*0.5, [8:16] = b
  tfAB   [P, 16, w] fp16  transformed planes, alpha(r,s,e)=4r+2s+e
                          (A in [0:8]) / beta(s,c,e)=4s+2c+e (B in [8:16])
  tp     [P, 32, w] fp16  products pi(r,s,e1,c,e2)=16r+8s+4e1+2c+e2
  tl     [P, 16, w] fp16  mu(r,s,h,c)=8r+4s+2h+c   (h=0 real, 1 imag)
  tC     [P, 8, w]  fp16  chi(r,e,c)=4r+2e+c
  toP    [P, 8, w]  fp16  output blade planes
  to     [P, w, 8]  fp32  interleaved output (ACT reinterleave)
"""

import sys

if "/opt/trn_rl_repo" not in sys.path:
    sys.path.insert(0, "/opt/trn_rl_repo")

import numpy as np

N_CORES = 8
P = 128  # SBUF partitions
N = 8    # blades

WIDTHS = (64, 192, 320, 384, 384, 384, 256, 64)  # sums to 2048 = npos_local / P

_module_cache = {}


def _canonical_cayley() -> np.ndarray:
    """Cl(3,0) geometric-product table, short-lex blade order (= reference)."""
    import itertools, functools, operator

    metric = [1, 1, 1]
    nv = len(metric)
    n = 2 ** nv
    basis = [1 << k for k in range(nv)]
    combos = itertools.chain.from_iterable(
        itertools.combinations(basis, r) for r in range(nv + 1))
    i2b = [functools.reduce(operator.or_, t, 0) for t in combos]
    b2i = {b: i for i, b in enumerate(i2b)}
    c = np.zeros((n, n, n), dtype=np.float32)
    for i, bi in enumerate(i2b):
        for j, bj in enumerate(i2b):
            a = bi >> 1
            s = 0
            while a:
                s += bin(a & bj).count("1")
                a >>= 1
            sign = -1.0 if (s & 1) else 1.0
            common = bi & bj
            k = 0
            while common:
                if common & 1:
                    sign *= metric[k]
                k += 1
                common >>= 1
            c[i, b2i[bi ^ bj], j] = sign
    return c


# ---------------- pauli fast path ----------------


def build_module_pauli(npos_local: int, widths):
    import concourse.bacc as bacc
    import concourse.mybir as mybir
    import concourse.tile as tile
    from concourse.bass import AP

    assert sum(widths) * P == npos_local
    f16 = mybir.dt.float16
    f32 = mybir.dt.float32
    ADD = mybir.AluOpType.add
    SUB = mybir.AluOpType.subtract
    MUL = mybir.AluOpType.mult

    nc = bacc.Bacc(None, target_bir_lowering=False, debug=False)
    with tile.TileContext(nc) as tc:
        with tc.tile_pool(name="dram", bufs=1, space="DRAM") as dram:
            a = dram.tile((npos_local, N), f32, kind="ExternalInput")
            b = dram.tile((npos_local, N), f32, kind="ExternalInput")
            out = dram.tile((npos_local, N), f32, kind="ExternalOutput")
            av = a[:].rearrange("(p f) n -> p (f n)", p=P)
            bv = b[:].rearrange("(p f) n -> p (f n)", p=P)
            ov = out[:].rearrange("(p f) n -> p (f n)", p=P)

            with (
                tc.tile_pool(name="io", bufs=3) as io_pool,
                tc.tile_pool(name="pl2", bufs=2) as pl2_pool,
                tc.tile_pool(name="pl", bufs=1) as pl_pool,
                tc.tile_pool(name="st", bufs=2) as st_pool,
                tc.tile_pool(name="wrm", bufs=1) as wrm_pool,
            ):
                # prewarm ScalarE's activation table (ACT_TABLE_LOAD ~2.6us)
                # before the first DMA lands so tile-0 deps don't pay for it
                warm = wrm_pool.tile([P, 2], f32, tag="warm")
                nc.vector.memset(warm[:, 0:1], 0.0)
                nc.scalar.copy(out=warm[:, 1:2], in_=warm[:, 0:1])

                def ap(t, pfree, off, dims):
                    base = t[:]
                    return AP(base.tensor, base.offset + off,
                              [[pfree, P]] + dims)

                pending_reint = []  # deferred reint+store closures, 1 tile lag

                pos0 = 0
                for t, w in enumerate(widths):
                    sl = slice(pos0 * N, (pos0 + w) * N)
                    pos0 += w
                    tab = io_pool.tile([P, 2 * w, N], f32, tag="tab")
                    tAB = pl2_pool.tile([P, 8, 2 * w], f16, tag="tAB")
                    tfAB = pl_pool.tile([P, 16, w], f16, tag="tfAB")
                    tl = pl_pool.tile([P, 16, w], f16, tag="tl")
                    tC = pl_pool.tile([P, 8, w], f16, tag="tC")
                    toP = pl2_pool.tile([P, 8, w], f16, tag="toP")

                    nc.sync.dma_start(
                        out=tab[:, 0:w, :].rearrange("p f n -> p (f n)"),
                        in_=av[:, sl])
                    nc.scalar.dma_start(
                        out=tab[:, w:2 * w, :].rearrange("p f n -> p (f n)"),
                        in_=bv[:, sl])

                    # single merged deinterleave + fp16 convert; tile 0 on
                    # DVE (otherwise idle during fill), rest on ScalarE.
                    # blade plane n holds [a_n (w) | b_n (w)]
                    last = t == len(widths) - 1
                    if t == 0:
                        # unscaled DVE deint during the fill; t0's reint
                        # carries the 0.5 on ACT instead
                        nc.vector.tensor_copy(
                            out=tAB[:], in_=tab[:].transpose([0, 2, 1]))
                    else:
                        # pre-scale a by 0.5 at the deint so every reint is
                        # a pure cast runnable on either engine
                        nc.scalar.mul(
                            out=tAB[:, :, 0:w],
                            in_=tab[:, 0:w, :].transpose([0, 2, 1]), mul=0.5)
                        nc.scalar.copy(
                            out=tAB[:, :, w:2 * w],
                            in_=tab[:, w:2 * w, :].transpose([0, 2, 1]))

                    # previous tile's reinterleave goes to ACT *after* this
                    # tile's deint so DVE never waits on ACT at boundaries
                    for fn in pending_reint:
                        fn()
                    pending_reint = []

                    # --- TF: 4 DVE ops [P,2,2,w] over (operand, pair, w) ---
                    # gamma: 0=M00r 1=M00i 2=M01r 3=M01i 4=M10r 5=M10i 6=M11r 7=M11i
                    # in tAB: blade n -> a at n*2w, b at n*2w + w
                    pf = 16 * w
                    for (oo, so, i0, s0, i1, s1, alu) in (
                        (0 * w, 1 * w, 0 * w, 8 * w, 6 * w, 8 * w, ADD),     # g0=m0+m3 g1=m4+m7
                        (4 * w, 1 * w, 2 * w, 2 * w, 10 * w, 2 * w, ADD),    # g4=m1+m5 g5=m2+m6
                        (2 * w, 4 * w, 2 * w, -2 * w, 10 * w, -4 * w, SUB),  # g2=m1-m5 g6=m0-m3
                        (3 * w, 4 * w, 12 * w, 2 * w, 4 * w, 4 * w, SUB),    # g3=m6-m2 g7=m7-m4
                    ):
                        nc.vector.tensor_tensor(
                            out=ap(tfAB, pf, oo,
                                   [[8 * w, 2], [so, 2], [1, w]]),
                            in0=ap(tAB, pf, i0,
                                   [[w, 2], [s0, 2], [1, w]]),
                            in1=ap(tAB, pf, i1,
                                   [[w, 2], [s1, 2], [1, w]]),
                            op=alu)

                    # --- PROD: 4 DVE ops; tp(r,s,e1,c,e2)=16r+8s+4e1+2c+e2 ---
                    tp = pl_pool.tile([P, 32, w], f16, tag="tp")
                    for r in (0, 1):
                        for s in (0, 1):
                            nc.vector.tensor_tensor(
                                out=ap(tp, 32 * w, (16 * r + 8 * s) * w,
                                       [[4 * w, 2], [w, 4], [1, w]]),
                                in0=ap(tfAB, pf, (4 * r + 2 * s) * w,
                                       [[w, 2], [0, 4], [1, w]]),
                                in1=ap(tfAB, pf, (8 + 4 * s) * w,
                                       [[0, 2], [w, 4], [1, w]]),
                                op=MUL)
                    # --- L1: 2 DVE ops [P,4,2,w] over ((r,s), c, w) ---
                    nc.vector.tensor_tensor(  # real: p(..00) - p(..11)
                        out=ap(tl, 16 * w, 0,
                               [[4 * w, 4], [w, 2], [1, w]]),
                        in0=ap(tp, 32 * w, 0,
                               [[8 * w, 4], [2 * w, 2], [1, w]]),
                        in1=ap(tp, 32 * w, 5 * w,
                               [[8 * w, 4], [2 * w, 2], [1, w]]),
                        op=SUB)
                    nc.vector.tensor_tensor(  # imag: p(..01) + p(..10)
                        out=ap(tl, 16 * w, 2 * w,
                               [[4 * w, 4], [w, 2], [1, w]]),
                        in0=ap(tp, 32 * w, 1 * w,
                               [[8 * w, 4], [2 * w, 2], [1, w]]),
                        in1=ap(tp, 32 * w, 4 * w,
                               [[8 * w, 4], [2 * w, 2], [1, w]]),
                        op=ADD)

                    # --- L2: 1 DVE op [P,2,4,w] over (r, (e,c), w) ---
                    nc.vector.tensor_tensor(
                        out=ap(tC, 8 * w, 0, [[4 * w, 2], [w, 4], [1, w]]),
                        in0=ap(tl, 16 * w, 0, [[8 * w, 2], [w, 4], [1, w]]),
                        in1=ap(tl, 16 * w, 4 * w,
                               [[8 * w, 2], [w, 4], [1, w]]),
                        op=ADD)

                    # --- OTF: 3 DVE ops -> blade planes ---
                    # o0=C0+C5 o1=C1+C4 o6=C6+C3 o7=C7+C2
                    nc.vector.tensor_tensor(
                        out=ap(toP, 8 * w, 0, [[6 * w, 2], [w, 2], [1, w]]),
                        in0=ap(tC, 8 * w, 0, [[6 * w, 2], [w, 2], [1, w]]),
                        in1=ap(tC, 8 * w, 5 * w,
                               [[-2 * w, 2], [-w, 2], [1, w]]),
                        op=ADD)
                    # o2=C6-C3 o3=C0-C5
                    nc.vector.tensor_tensor(
                        out=ap(toP, 8 * w, 2 * w, [[w, 2], [1, w]]),
                        in0=ap(tC, 8 * w, 6 * w, [[-6 * w, 2], [1, w]]),
                        in1=ap(tC, 8 * w, 3 * w, [[2 * w, 2], [1, w]]),
                        op=SUB)
                    # o4=C2-C7 o5=C4-C1
                    nc.vector.tensor_tensor(
                        out=ap(toP, 8 * w, 4 * w, [[w, 2], [1, w]]),
                        in0=ap(tC, 8 * w, 2 * w, [[2 * w, 2], [1, w]]),
                        in1=ap(tC, 8 * w, 7 * w, [[-6 * w, 2], [1, w]]),
                        op=SUB)

                    # reinterleave halves + fp32 convert + 0.5 scale on
                    # ScalarE; deferred one tile so ACT runs deint first
                    # reints: t0 = scaled on ACT (its deint was unscaled);
                    # t1 and the two tail tiles = pure DVE casts in windows
                    # where DVE would idle; the rest = pure casts on ACT
                    on_dve = t in (1, len(widths) - 1)

                    def make_reint(toP=toP, sl=sl, w=w, t=t, on_dve=on_dve):
                        def emit():
                            h = w // 2
                            for (c0, cn) in ((0, h), (h, w - h)):
                                toH = st_pool.tile([P, cn, N], f32,
                                                   tag=f"to{c0 != 0:d}")
                                src = toP[:, :, c0:c0 + cn].transpose(
                                    [0, 2, 1])
                                if t == 0:
                                    nc.scalar.mul(out=toH[:], in_=src,
                                                  mul=0.5)
                                elif on_dve:
                                    nc.vector.tensor_copy(out=toH[:], in_=src)
                                else:
                                    nc.scalar.copy(out=toH[:], in_=src)
                                nc.sync.dma_start(
                                    out=ov[:, sl][:, c0 * N:(c0 + cn) * N],
                                    in_=toH[:].rearrange("p f n -> p (f n)"))
                        return emit

                    if t == len(widths) - 1:
                        make_reint()()
                    else:
                        pending_reint.append(make_reint())
                for fn in pending_reint:
                    fn()
    nc.compile()
    return nc, a.name, b.name, out.name


# ---------------- generic fallback (any cayley) ----------------


def _terms_by_j(cayley: np.ndarray):
    terms = [[] for _ in range(N)]
    for i in range(N):
        for j in range(N):
            for k in range(N):
                v = float(cayley[i, j, k])
                if v != 0.0:
                    terms[j].append((i, k, v))
    return terms


def _build_module(npos_local: int, terms):
    import concourse.bacc as bacc
    import concourse.mybir as mybir
    import concourse.tile as tile

    W = 256
    assert npos_local % (P * W) == 0
    T = npos_local // (P * W)
    fast = all(len(t) == 8 for t in terms)

    nc = bacc.Bacc(None, target_bir_lowering=False, debug=False)
    with tile.TileContext(nc) as tc:
        with tc.tile_pool(name="dram", bufs=1, space="DRAM") as dram:
            a = dram.tile((npos_local, N), mybir.dt.float32, kind="ExternalInput")
            b = dram.tile((npos_local, N), mybir.dt.float32, kind="ExternalInput")
            out = dram.tile((npos_local, N), mybir.dt.float32, kind="ExternalOutput")
            av = a[:].rearrange("(p f) n -> p (f n)", p=P)
            bv = b[:].rearrange("(p f) n -> p (f n)", p=P)
            ov = out[:].rearrange("(p f) n -> p (f n)", p=P)
            with (
                tc.tile_pool(name="io", bufs=4) as io_pool,
                tc.tile_pool(name="prod", bufs=1) as prod_pool,
            ):
                for t in range(T):
                    sl = slice(t * W * N, (t + 1) * W * N)
                    ta = io_pool.tile([P, W, N], mybir.dt.float32, tag="ta")
                    tb = io_pool.tile([P, W, N], mybir.dt.float32, tag="tb")
                    to = io_pool.tile([P, W, N], mybir.dt.float32, tag="to")
                    nc.sync.dma_start(
                        out=ta[:].rearrange("p f n -> p (f n)"), in_=av[:, sl]
                    )
                    nc.sync.dma_start(
                        out=tb[:].rearrange("p f n -> p (f n)"), in_=bv[:, sl]
                    )
                    if fast:
                        p0 = prod_pool.tile([P, 64, W], mybir.dt.float32, tag="p0")
                        p1 = prod_pool.tile([P, 32, W], mybir.dt.float32, tag="p1")
                        p2 = prod_pool.tile([P, 16, W], mybir.dt.float32, tag="p2")
                        for j in range(N):
                            for l, (i, k, v) in enumerate(terms[j]):
                                nc.vector.scalar_tensor_tensor(
                                    out=p0[:, j * 8 + l, :],
                                    in0=ta[:, :, i],
                                    scalar=v,
                                    in1=tb[:, :, k],
                                    op0=mybir.AluOpType.mult,
                                    op1=mybir.AluOpType.mult,
                                )
                        nc.vector.tensor_tensor(
                            out=p1[:], in0=p0[:, 0::2, :], in1=p0[:, 1::2, :],
                            op=mybir.AluOpType.add,
                        )
                        nc.vector.tensor_tensor(
                            out=p2[:], in0=p1[:, 0::2, :], in1=p1[:, 1::2, :],
                            op=mybir.AluOpType.add,
                        )
                        nc.vector.tensor_tensor(
                            out=to[:].transpose([0, 2, 1]),
                            in0=p2[:, 0::2, :], in1=p2[:, 1::2, :],
                            op=mybir.AluOpType.add,
                        )
                    else:
                        pa = prod_pool.tile([P, W], mybir.dt.float32, tag="pa")
                        acc = prod_pool.tile([P, W], mybir.dt.float32, tag="acc")
                        for j in range(N):
                            if not terms[j]:
                                nc.vector.memset(to[:, :, j], 0.0)
                                continue
                            i, k, v = terms[j][0]
                            nc.vector.scalar_tensor_tensor(
                                out=acc[:], in0=ta[:, :, i], scalar=v,
                                in1=tb[:, :, k],
                                op0=mybir.AluOpType.mult, op1=mybir.AluOpType.mult,
                            )
                            for (i, k, v) in terms[j][1:]:
                                nc.vector.scalar_tensor_tensor(
                                    out=pa[:], in0=ta[:, :, i], scalar=v,
                                    in1=tb[:, :, k],
                                    op0=mybir.AluOpType.mult, op1=mybir.AluOpType.mult,
                                )
                                nc.vector.tensor_tensor(
                                    out=acc[:], in0=acc[:], in1=pa[:],
                                    op=mybir.AluOpType.add,
                                )
                            nc.vector.tensor_copy(out=to[:, :, j], in_=acc[:])
                    nc.sync.dma_start(
                        out=ov[:, sl], in_=to[:].rearrange("p f n -> p (f n)")
                    )
    nc.compile()
    return nc, a.name, b.name, out.name


def _get_module(npos_local: int, cayley: np.ndarray):
    key = (npos_local, cayley.tobytes())
    if key not in _module_cache:
        if (npos_local % P == 0 and sum(WIDTHS) * P == npos_local
                and np.array_equal(cayley, _canonical_cayley())):
            _module_cache[key] = build_module_pauli(npos_local, WIDTHS)
        else:
            _module_cache[key] = _build_module(npos_local, _terms_by_j(cayley))
    return _module_cache[key]


def _run(inputs: dict, trace: bool = False, tmpdir=None):
    a = np.asarray(inputs["a"], dtype=np.float32)
    b = np.asarray(inputs["b"], dtype=np.float32)
    cayley = np.asarray(inputs["cayley"], dtype=np.float32)
    B, S, NN = a.shape
    assert NN == N and b.shape == a.shape and cayley.shape == (N, N, N)
    assert B % N_CORES == 0
    nb = B // N_CORES
    npos_local = nb * S

    nc, a_name, b_name, out_name = _get_module(npos_local, cayley)

    a_sh = a.reshape(N_CORES, npos_local, N)
    b_sh = b.reshape(N_CORES, npos_local, N)
    in_maps = [
        {a_name: np.ascontiguousarray(a_sh[c]), b_name: np.ascontiguousarray(b_sh[c])}
        for c in range(N_CORES)
    ]

    from concourse import bass_utils

    kwargs = {}
    if trace:
        _install_ntff_shim()
        bass_utils.upload_artifacts = lambda d: f"local:{d}"
        kwargs = {"trace": True, "tmpdir": tmpdir}
    res = bass_utils.run_bass_kernel_spmd(
        nc, in_maps, core_ids=list(range(N_CORES)), **kwargs
    )
    out = np.concatenate(
        [res.results[c][out_name].reshape(1, nb, S, N) for c in range(N_CORES)], axis=0
    ).reshape(B, S, N)
    return out, res


def kernel(**inputs) -> np.ndarray:
    out, _ = _run(inputs, trace=False)
    return out


def kernel_traced(**inputs):
    """Run with NTFF profiling; returns (out, exec_time_ns, trace_path)."""
    import tempfile

    out, res = _run(inputs, trace=True, tmpdir=tempfile.mkdtemp(prefix="gp_trace_"))
    trace_path = res.instructions_and_trace[1] if res.instructions_and_trace else None
    return out, res.exec_time_ns, trace_path


def _install_ntff_shim():
    """Provide antenv.axon_hooks with an NTFF profile hook if missing."""
    try:
        from antenv.axon_hooks import get_axon_ntff_profile_hook  # noqa: F401

        return
    except ImportError:
        pass
    import types, ctypes, contextlib

    holder = {"hook": None}
    mod = types.ModuleType("antenv.axon_hooks")
    mod.set_axon_ntff_profile_hook = lambda h: holder.__setitem__("hook", h)
    mod.get_axon_ntff_profile_hook = lambda: holder["hook"]
    sys.modules["antenv.axon_hooks"] = mod

    so_path = "/opt/axon/libaxon_pjrt.so"
    try:
        lib = ctypes.CDLL(so_path)
        if not hasattr(lib, "axon_start_nrt_profile"):
            return
    except OSError:
        return
    lib.axon_start_nrt_profile.argtypes = [
        ctypes.POINTER(ctypes.c_int64),
        ctypes.c_size_t,
    ]
    lib.axon_start_nrt_profile.restype = ctypes.c_int64
    lib.axon_stop_nrt_profile.argtypes = [ctypes.c_char_p]
    lib.axon_stop_nrt_profile.restype = ctypes.c_int64

    @contextlib.contextmanager
    def _hook(output_dir, device_ids):
        import jax

        jax.devices()
        if device_ids:
            ids = (ctypes.c_int64 * len(device_ids))(*device_ids)
            rc = lib.axon_start_nrt_profile(ids, len(device_ids))
        else:
            rc = lib.axon_start_nrt_profile(None, 0)
        if rc != 0:
            raise RuntimeError(f"axon_start_nrt_profile rc={rc}")
        try:
            yield
        finally:
            n = lib.axon_stop_nrt_profile(str(output_dir).encode())
            print(f"profile: {n} file(s) written to {output_dir}", file=sys.stderr)

    mod.set_axon_ntff_profile_hook(_hook)


# revision 30
# speedup vs baseline: 1.2466x; 1.0048x over previous
"""Trainium2 Bass kernel for the Clifford-algebra geometric product.

  out[..., j] = sum_{i,k} a[..., i] * cayley[i, j, k] * b[..., k]

Full inputs a, b: (2048, 1024, 8) fp32, cayley: (8, 8, 8) fp32.
Sharding: pure data parallelism over the leading batch axis across 8
NeuronCores.

Fast path ("pauli"): Cl(3,0) is isomorphic to the 2x2 complex matrix
algebra M2(C) (Pauli matrices).  Writing each multivector as
  M = [[ (a0+a3) + i(a12+a123), (a1-a13) + i(a23-a2) ],
       [ (a1+a13) + i(a2+a23),  (a0-a3) + i(a123-a12) ]]
the geometric product is the 2x2 complex matmul C = A*B, and the output
coefficients are (sum/difference pairs of C entries)/2.  This cuts the
elementwise work from 120 ops/position (64 products + 56 tree adds) to
80 ops/position (16 transform + 32 products + 24 combine + 8 out), all
expressible as plain tensor_tensor adds/mults.

All compute runs in fp16 on the DVE at 2 elem/lane/cycle (the 2x_1p
packed mode; fp32 TT runs at 1x).  ScalarE (ACT) does the
interleaved<->plane transposes with the fp32<->fp16 conversion and the
0.5 scale folded in (transposed ACT copies cost the same with or
without scale/convert).  Measured end-to-end fp16 error vs the fp32
reference: ~8e-4 max-rel (gate is 2e-2).

Layout per tile of width w positions/partition:
  ta/tb  [P, w, 8] fp32   interleaved (contiguous DMA)
  tAB    [P, 16, w] fp16  blade planes: [0:8] = a*0.5, [8:16] = b
  tfAB   [P, 16, w] fp16  transformed planes, alpha(r,s,e)=4r+2s+e
                          (A in [0:8]) / beta(s,c,e)=4s+2c+e (B in [8:16])
  tp     [P, 32, w] fp16  products pi(r,s,e1,c,e2)=16r+8s+4e1+2c+e2
  tl     [P, 16, w] fp16  mu(r,s,h,c)=8r+4s+2h+c   (h=0 real, 1 imag)
  tC     [P, 8, w]  fp16  chi(r,e,c)=4r+2e+c
  toP    [P, 8, w]  fp16  output blade planes
  to     [P, w, 8]  fp32  interleaved output (ACT reinterleave)
"""

import sys

if "/opt/trn_rl_repo" not in sys.path:
    sys.path.insert(0, "/opt/trn_rl_repo")

import numpy as np

N_CORES = 8
P = 128  # SBUF partitions
N = 8    # blades

WIDTHS = (64, 192, 320, 384, 384, 384, 256, 64)  # sums to 2048 = npos_local / P

_module_cache = {}


def _canonical_cayley() -> np.ndarray:
    """Cl(3,0) geometric-product table, short-lex blade order (= reference)."""
    import itertools, functools, operator

    metric = [1, 1, 1]
    nv = len(metric)
    n = 2 ** nv
    basis = [1 << k for k in range(nv)]
    combos = itertools.chain.from_iterable(
        itertools.combinations(basis, r) for r in range(nv + 1))
    i2b = [functools.reduce(operator.or_, t, 0) for t in combos]
    b2i = {b: i for i, b in enumerate(i2b)}
    c = np.zeros((n, n, n), dtype=np.float32)
    for i, bi in enumerate(i2b):
        for j, bj in enumerate(i2b):
            a = bi >> 1
            s = 0
            while a:
                s += bin(a & bj).count("1")
                a >>= 1
            sign = -1.0 if (s & 1) else 1.0
            common = bi & bj
            k = 0
            while common:
                if common & 1:
                    sign *= metric[k]
                k += 1
                common >>= 1
            c[i, b2i[bi ^ bj], j] = sign
    return c


# ---------------- pauli fast path ----------------


def build_module_pauli(npos_local: int, widths):
    import concourse.bacc as bacc
    import concourse.mybir as mybir
    import concourse.tile as tile
    from concourse.bass import AP

    assert sum(widths) * P == npos_local
    f16 = mybir.dt.float16
    f32 = mybir.dt.float32
    ADD = mybir.AluOpType.add
    SUB = mybir.AluOpType.subtract
    MUL = mybir.AluOpType.mult

    nc = bacc.Bacc(None, target_bir_lowering=False, debug=False)
    with tile.TileContext(nc) as tc:
        with tc.tile_pool(name="dram", bufs=1, space="DRAM") as dram:
            a = dram.tile((npos_local, N), f32, kind="ExternalInput")
            b = dram.tile((npos_local, N), f32, kind="ExternalInput")
            out = dram.tile((npos_local, N), f32, kind="ExternalOutput")
            av = a[:].rearrange("(p f) n -> p (f n)", p=P)
            bv = b[:].rearrange("(p f) n -> p (f n)", p=P)
            ov = out[:].rearrange("(p f) n -> p (f n)", p=P)

            with (
                tc.tile_pool(name="io", bufs=3) as io_pool,
                tc.tile_pool(name="pl2", bufs=2) as pl2_pool,
                tc.tile_pool(name="pl", bufs=1) as pl_pool,
                tc.tile_pool(name="st", bufs=2) as st_pool,
                tc.tile_pool(name="wrm", bufs=1) as wrm_pool,
            ):
                # prewarm ScalarE's activation table (ACT_TABLE_LOAD ~2.6us)
                # before the first DMA lands so tile-0 deps don't pay for it
                warm = wrm_pool.tile([P, 2], f32, tag="warm")
                nc.vector.memset(warm[:, 0:1], 0.0)
                nc.scalar.copy(out=warm[:, 1:2], in_=warm[:, 0:1])

                def ap(t, pfree, off, dims):
                    base = t[:]
                    return AP(base.tensor, base.offset + off,
                              [[pfree, P]] + dims)

                pending_reint = []  # deferred reint+store closures, 1 tile lag

                pos0 = 0
                for t, w in enumerate(widths):
                    sl = slice(pos0 * N, (pos0 + w) * N)
                    pos0 += w
                    tab = io_pool.tile([P, 2 * w, N], f32, tag="tab")
                    tAB = pl2_pool.tile([P, 8, 2 * w], f16, tag="tAB")
                    tfAB = pl_pool.tile([P, 16, w], f16, tag="tfAB")
                    tl = pl_pool.tile([P, 16, w], f16, tag="tl")
                    tC = pl_pool.tile([P, 8, w], f16, tag="tC")
                    toP = pl2_pool.tile([P, 8, w], f16, tag="toP")

                    nc.sync.dma_start(
                        out=tab[:, 0:w, :].rearrange("p f n -> p (f n)"),
                        in_=av[:, sl])
                    # ACT's queue starts ~5us after SP's at program
                    # bringup, so the first tiles' b-loads go via SP too
                    bq = nc.sync if t <= 2 else nc.scalar
                    bq.dma_start(
                        out=tab[:, w:2 * w, :].rearrange("p f n -> p (f n)"),
                        in_=bv[:, sl])

                    # single merged deinterleave + fp16 convert; tile 0 on
                    # DVE (otherwise idle during fill), rest on ScalarE.
                    # blade plane n holds [a_n (w) | b_n (w)]
                    last = t == len(widths) - 1
                    if t == 0:
                        # unscaled DVE deint during the fill; t0's reint
                        # carries the 0.5 on ACT instead
                        nc.vector.tensor_copy(
                            out=tAB[:], in_=tab[:].transpose([0, 2, 1]))
                    else:
                        # pre-scale a by 0.5 at the deint so every reint is
                        # a pure cast runnable on either engine
                        nc.scalar.mul(
                            out=tAB[:, :, 0:w],
                            in_=tab[:, 0:w, :].transpose([0, 2, 1]), mul=0.5)
                        nc.scalar.copy(
                            out=tAB[:, :, w:2 * w],
                            in_=tab[:, w:2 * w, :].transpose([0, 2, 1]))

                    # previous tile's reinterleave goes to ACT *after* this
                    # tile's deint so DVE never waits on ACT at boundaries
                    for fn in pending_reint:
                        fn()
                    pending_reint = []

                    # --- TF: 4 DVE ops [P,2,2,w] over (operand, pair, w) ---
                    # gamma: 0=M00r 1=M00i 2=M01r 3=M01i 4=M10r 5=M10i 6=M11r 7=M11i
                    # in tAB: blade n -> a at n*2w, b at n*2w + w
                    pf = 16 * w
                    for (oo, so, i0, s0, i1, s1, alu) in (
                        (0 * w, 1 * w, 0 * w, 8 * w, 6 * w, 8 * w, ADD),     # g0=m0+m3 g1=m4+m7
                        (4 * w, 1 * w, 2 * w, 2 * w, 10 * w, 2 * w, ADD),    # g4=m1+m5 g5=m2+m6
                        (2 * w, 4 * w, 2 * w, -2 * w, 10 * w, -4 * w, SUB),  # g2=m1-m5 g6=m0-m3
                        (3 * w, 4 * w, 12 * w, 2 * w, 4 * w, 4 * w, SUB),    # g3=m6-m2 g7=m7-m4
                    ):
                        nc.vector.tensor_tensor(
                            out=ap(tfAB, pf, oo,
                                   [[8 * w, 2], [so, 2], [1, w]]),
                            in0=ap(tAB, pf, i0,
                                   [[w, 2], [s0, 2], [1, w]]),
                            in1=ap(tAB, pf, i1,
                                   [[w, 2], [s1, 2], [1, w]]),
                            op=alu)

                    # --- PROD: 4 DVE ops; tp(r,s,e1,c,e2)=16r+8s+4e1+2c+e2 ---
                    tp = pl_pool.tile([P, 32, w], f16, tag="tp")
                    for r in (0, 1):
                        for s in (0, 1):
                            nc.vector.tensor_tensor(
                                out=ap(tp, 32 * w, (16 * r + 8 * s) * w,
                                       [[4 * w, 2], [w, 4], [1, w]]),
                                in0=ap(tfAB, pf, (4 * r + 2 * s) * w,
                                       [[w, 2], [0, 4], [1, w]]),
                                in1=ap(tfAB, pf, (8 + 4 * s) * w,
                                       [[0, 2], [w, 4], [1, w]]),
                                op=MUL)
                    # --- L1: 2 DVE ops [P,4,2,w] over ((r,s), c, w) ---
                    nc.vector.tensor_tensor(  # real: p(..00) - p(..11)
                        out=ap(tl, 16 * w, 0,
                               [[4 * w, 4], [w, 2], [1, w]]),
                        in0=ap(tp, 32 * w, 0,
                               [[8 * w, 4], [2 * w, 2], [1, w]]),
                        in1=ap(tp, 32 * w, 5 * w,
                               [[8 * w, 4], [2 * w, 2], [1, w]]),
                        op=SUB)
                    nc.vector.tensor_tensor(  # imag: p(..01) + p(..10)
                        out=ap(tl, 16 * w, 2 * w,
                               [[4 * w, 4], [w, 2], [1, w]]),
                        in0=ap(tp, 32 * w, 1 * w,
                               [[8 * w, 4], [2 * w, 2], [1, w]]),
                        in1=ap(tp, 32 * w, 4 * w,
                               [[8 * w, 4], [2 * w, 2], [1, w]]),
                        op=ADD)

                    # --- L2: 1 DVE op [P,2,4,w] over (r, (e,c), w) ---
                    nc.vector.tensor_tensor(
                        out=ap(tC, 8 * w, 0, [[4 * w, 2], [w, 4], [1, w]]),
                        in0=ap(tl, 16 * w, 0, [[8 * w, 2], [w, 4], [1, w]]),
                        in1=ap(tl, 16 * w, 4 * w,
                               [[8 * w, 2], [w, 4], [1, w]]),
                        op=ADD)

                    # --- OTF: 3 DVE ops -> blade planes ---
                    # o0=C0+C5 o1=C1+C4 o6=C6+C3 o7=C7+C2
                    nc.vector.tensor_tensor(
                        out=ap(toP, 8 * w, 0, [[6 * w, 2], [w, 2], [1, w]]),
                        in0=ap(tC, 8 * w, 0, [[6 * w, 2], [w, 2], [1, w]]),
                        in1=ap(tC, 8 * w, 5 * w,
                               [[-2 * w, 2], [-w, 2], [1, w]]),
                        op=ADD)
                    # o2=C6-C3 o3=C0-C5
                    nc.vector.tensor_tensor(
                        out=ap(toP, 8 * w, 2 * w, [[w, 2], [1, w]]),
                        in0=ap(tC, 8 * w, 6 * w, [[-6 * w, 2], [1, w]]),
                        in1=ap(tC, 8 * w, 3 * w, [[2 * w, 2], [1, w]]),
                        op=SUB)
                    # o4=C2-C7 o5=C4-C1
                    nc.vector.tensor_tensor(
                        out=ap(toP, 8 * w, 4 * w, [[w, 2], [1, w]]),
                        in0=ap(tC, 8 * w, 2 * w, [[2 * w, 2], [1, w]]),
                        in1=ap(tC, 8 * w, 7 * w, [[-6 * w, 2], [1, w]]),
                        op=SUB)

                    # reinterleave halves + fp32 convert + 0.5 scale on
                    # ScalarE; deferred one tile so ACT runs deint first
                    # reints: t0 = scaled on ACT (its deint was unscaled);
                    # t1 and the two tail tiles = pure DVE casts in windows
                    # where DVE would idle; the rest = pure casts on ACT
                    on_dve = t in (1, len(widths) - 1)

                    def make_reint(toP=toP, sl=sl, w=w, t=t, on_dve=on_dve):
                        def emit():
                            h = w // 2
                            for (c0, cn) in ((0, h), (h, w - h)):
                                toH = st_pool.tile([P, cn, N], f32,
                                                   tag=f"to{c0 != 0:d}")
                                src = toP[:, :, c0:c0 + cn].transpose(
                                    [0, 2, 1])
                                if t == 0:
                                    nc.scalar.mul(out=toH[:], in_=src,
                                                  mul=0.5)
                                elif on_dve:
                                    nc.vector.tensor_copy(out=toH[:], in_=src)
                                else:
                                    nc.scalar.copy(out=toH[:], in_=src)
                                nc.sync.dma_start(
                                    out=ov[:, sl][:, c0 * N:(c0 + cn) * N],
                                    in_=toH[:].rearrange("p f n -> p (f n)"))
                        return emit

                    if t == len(widths) - 1:
                        make_reint()()
                    else:
                        pending_reint.append(make_reint())
                for fn in pending_reint:
                    fn()
    nc.compile()
    return nc, a.name, b.name, out.name


# ---------------- generic fallback (any cayley) ----------------


def _terms_by_j(cayley: np.ndarray):
    terms = [[] for _ in range(N)]
    for i in range(N):
        for j in range(N):
            for k in range(N):
                v = float(cayley[i, j, k])
                if v != 0.0:
                    terms[j].append((i, k, v))
    return terms


def _build_module(npos_local: int, terms):
    import concourse.bacc as bacc
    import concourse.mybir as mybir
    import concourse.tile as tile

    W = 256
    assert npos_local % (P * W) == 0
    T = npos_local // (P * W)
    fast = all(len(t) == 8 for t in terms)

    nc = bacc.Bacc(None, target_bir_lowering=False, debug=False)
    with tile.TileContext(nc) as tc:
        with tc.tile_pool(name="dram", bufs=1, space="DRAM") as dram:
            a = dram.tile((npos_local, N), mybir.dt.float32, kind="ExternalInput")
            b = dram.tile((npos_local, N), mybir.dt.float32, kind="ExternalInput")
            out = dram.tile((npos_local, N), mybir.dt.float32, kind="ExternalOutput")
            av = a[:].rearrange("(p f) n -> p (f n)", p=P)
            bv = b[:].rearrange("(p f) n -> p (f n)", p=P)
            ov = out[:].rearrange("(p f) n -> p (f n)", p=P)
            with (
                tc.tile_pool(name="io", bufs=4) as io_pool,
                tc.tile_pool(name="prod", bufs=1) as prod_pool,
            ):
                for t in range(T):
                    sl = slice(t * W * N, (t + 1) * W * N)
                    ta = io_pool.tile([P, W, N], mybir.dt.float32, tag="ta")
                    tb = io_pool.tile([P, W, N], mybir.dt.float32, tag="tb")
                    to = io_pool.tile([P, W, N], mybir.dt.float32, tag="to")
                    nc.sync.dma_start(
                        out=ta[:].rearrange("p f n -> p (f n)"), in_=av[:, sl]
                    )
                    nc.sync.dma_start(
                        out=tb[:].rearrange("p f n -> p (f n)"), in_=bv[:, sl]
                    )
                    if fast:
                        p0 = prod_pool.tile([P, 64, W], mybir.dt.float32, tag="p0")
                        p1 = prod_pool.tile([P, 32, W], mybir.dt.float32, tag="p1")
                        p2 = prod_pool.tile([P, 16, W], mybir.dt.float32, tag="p2")
                        for j in range(N):
                            for l, (i, k, v) in enumerate(terms[j]):
                                nc.vector.scalar_tensor_tensor(
                                    out=p0[:, j * 8 + l, :],
                                    in0=ta[:, :, i],
                                    scalar=v,
                                    in1=tb[:, :, k],
                                    op0=mybir.AluOpType.mult,
                                    op1=mybir.AluOpType.mult,
                                )
                        nc.vector.tensor_tensor(
                            out=p1[:], in0=p0[:, 0::2, :], in1=p0[:, 1::2, :],
                            op=mybir.AluOpType.add,
                        )
                        nc.vector.tensor_tensor(
                            out=p2[:], in0=p1[:, 0::2, :], in1=p1[:, 1::2, :],
                            op=mybir.AluOpType.add,
                        )
                        nc.vector.tensor_tensor(
                            out=to[:].transpose([0, 2, 1]),
                            in0=p2[:, 0::2, :], in1=p2[:, 1::2, :],
                            op=mybir.AluOpType.add,
                        )
                    else:
                        pa = prod_pool.tile([P, W], mybir.dt.float32, tag="pa")
                        acc = prod_pool.tile([P, W], mybir.dt.float32, tag="acc")
                        for j in range(N):
                            if not terms[j]:
                                nc.vector.memset(to[:, :, j], 0.0)
                                continue
                            i, k, v = terms[j][0]
                            nc.vector.scalar_tensor_tensor(
                                out=acc[:], in0=ta[:, :, i], scalar=v,
                                in1=tb[:, :, k],
                                op0=mybir.AluOpType.mult, op1=mybir.AluOpType.mult,
                            )
                            for (i, k, v) in terms[j][1:]:
                                nc.vector.scalar_tensor_tensor(
                                    out=pa[:], in0=ta[:, :, i], scalar=v,
                                    in1=tb[:, :, k],
                                    op0=mybir.AluOpType.mult, op1=mybir.AluOpType.mult,
                                )
                                nc.vector.tensor_tensor(
                                    out=acc[:], in0=acc[:], in1=pa[:],
                                    op=mybir.AluOpType.add,
                                )
                            nc.vector.tensor_copy(out=to[:, :, j], in_=acc[:])
                    nc.sync.dma_start(
                        out=ov[:, sl], in_=to[:].rearrange("p f n -> p (f n)")
                    )
    nc.compile()
    return nc, a.name, b.name, out.name


def _get_module(npos_local: int, cayley: np.ndarray):
    key = (npos_local, cayley.tobytes())
    if key not in _module_cache:
        if (npos_local % P == 0 and sum(WIDTHS) * P == npos_local
                and np.array_equal(cayley, _canonical_cayley())):
            _module_cache[key] = build_module_pauli(npos_local, WIDTHS)
        else:
            _module_cache[key] = _build_module(npos_local, _terms_by_j(cayley))
    return _module_cache[key]


def _run(inputs: dict, trace: bool = False, tmpdir=None):
    a = np.asarray(inputs["a"], dtype=np.float32)
    b = np.asarray(inputs["b"], dtype=np.float32)
    cayley = np.asarray(inputs["cayley"], dtype=np.float32)
    B, S, NN = a.shape
    assert NN == N and b.shape == a.shape and cayley.shape == (N, N, N)
    assert B % N_CORES == 0
    nb = B // N_CORES
    npos_local = nb * S

    nc, a_name, b_name, out_name = _get_module(npos_local, cayley)

    a_sh = a.reshape(N_CORES, npos_local, N)
    b_sh = b.reshape(N_CORES, npos_local, N)
    in_maps = [
        {a_name: np.ascontiguousarray(a_sh[c]), b_name: np.ascontiguousarray(b_sh[c])}
        for c in range(N_CORES)
    ]

    from concourse import bass_utils

    kwargs = {}
    if trace:
        _install_ntff_shim()
        bass_utils.upload_artifacts = lambda d: f"local:{d}"
        kwargs = {"trace": True, "tmpdir": tmpdir}
    res = bass_utils.run_bass_kernel_spmd(
        nc, in_maps, core_ids=list(range(N_CORES)), **kwargs
    )
    out = np.concatenate(
        [res.results[c][out_name].reshape(1, nb, S, N) for c in range(N_CORES)], axis=0
    ).reshape(B, S, N)
    return out, res


def kernel(**inputs) -> np.ndarray:
    out, _ = _run(inputs, trace=False)
    return out


def kernel_traced(**inputs):
    """Run with NTFF profiling; returns (out, exec_time_ns, trace_path)."""
    import tempfile

    out, res = _run(inputs, trace=True, tmpdir=tempfile.mkdtemp(prefix="gp_trace_"))
    trace_path = res.instructions_and_trace[1] if res.instructions_and_trace else None
    return out, res.exec_time_ns, trace_path


def _install_ntff_shim():
    """Provide antenv.axon_hooks with an NTFF profile hook if missing."""
    try:
        from antenv.axon_hooks import get_axon_ntff_profile_hook  # noqa: F401

        return
    except ImportError:
        pass
    import types, ctypes, contextlib

    holder = {"hook": None}
    mod = types.ModuleType("antenv.axon_hooks")
    mod.set_axon_ntff_profile_hook = lambda h: holder.__setitem__("hook", h)
    mod.get_axon_ntff_profile_hook = lambda: holder["hook"]
    sys.modules["antenv.axon_hooks"] = mod

    so_path = "/opt/axon/libaxon_pjrt.so"
    try:
        lib = ctypes.CDLL(so_path)
        if not hasattr(lib, "axon_start_nrt_profile"):
            return
    except OSError:
        return
    lib.axon_start_nrt_profile.argtypes = [
        ctypes.POINTER(ctypes.c_int64),
        ctypes.c_size_t,
    ]
    lib.axon_start_nrt_profile.restype = ctypes.c_int64
    lib.axon_stop_nrt_profile.argtypes = [ctypes.c_char_p]
    lib.axon_stop_nrt_profile.restype = ctypes.c_int64

    @contextlib.contextmanager
    def _hook(output_dir, device_ids):
        import jax

        jax.devices()
        if device_ids:
            ids = (ctypes.c_int64 * len(device_ids))(*device_ids)
            rc = lib.axon_start_nrt_profile(ids, len(device_ids))
        else:
            rc = lib.axon_start_nrt_profile(None, 0)
        if rc != 0:
            raise RuntimeError(f"axon_start_nrt_profile rc={rc}")
        try:
            yield
        finally:
            n = lib.axon_stop_nrt_profile(str(output_dir).encode())
            print(f"profile: {n} file(s) written to {output_dir}", file=sys.stderr)

    mod.set_axon_ntff_profile_hook(_hook)
